# revision 43
# baseline (speedup 1.0000x reference)
"""Trainium2 Bass kernel for nn_CausalLayer (bilinear causal mixing layer).

Math (per batch b):
    E = ae[x]                                # [L, D] gather
    S[i,j] = E_i @ w @ E_j                   # bilinear pairwise score
    coef[i,j] = (i+1)/(j+1) for i<j else 0
    res[:,j] = bx[:,j] + sum_i coef[i,j]*S[i,j]*bx[:,i]

Rather than materializing the [L, L] score matrix (O(L^2 H) flops), we use the
chunked linear-attention identity. With a_i = w^T E_i and y_i = (i+1)*bx_i:

    res_j = bx_j + (1/(j+1)) * [ M_cj @ E_j + sum_{i<j, same chunk} (a_i.E_j) y_i ]
    M_c   = sum_{i in chunks < c} y_i a_i^T      (rank-D running state, [D, H])

Per 128-token chunk that is: a few tiny [*,64/128] matmuls, one masked [128,128]
score block, and three [*,768] matmuls -- O(L*C*(D+H) + L*D*H) total, 16x fewer
flops than the reference einsum, which puts the kernel at the HBM roofline
(bf16 bx in + f32 res out + gathers ~= 21 MB/core).

Sharding: batch-parallel, 2 of 16 batches per NeuronCore across 8 cores; ae/w
and the small constant tables are replicated. No cross-core communication.
"""

import os
import sys

for _p in ("/opt/trn_rl_repo", "/root/.axon_site/_ro/trn_rl_repo"):
    if os.path.isdir(_p) and _p not in sys.path:
        sys.path.insert(0, _p)

import numpy as np

B, L, H = 16, 2048, 768
V, D = 30000, 64
NCORES = 8
BPC = B // NCORES          # batches per core
C = 128                    # chunk (tile) size along sequence
NCH = L // C               # chunks per batch
ROWS = BPC * L             # bx rows per core

# dtype for the matmul path. This build is tuned for "bf16" (the fused gather
# table and transposes are bf16); measured scale-relative absmax error vs the
# fp32 reference is ~3.3e-3 with fp32 PSUM accumulation throughout.
BIG_DT = "bf16"

_compiled = {}


def _np_consts():
    i = np.arange(C, dtype=np.float64)
    cmask = np.zeros((C, NCH * C), np.float32)
    consts = np.zeros((C, 2 * NCH), np.float32)
    for c in range(NCH):
        gi = c * C + i
        cmask[:, c * C:(c + 1) * C] = np.where(
            i[:, None] < i[None, :], (gi + 1.0)[:, None], 0.0
        ).astype(np.float32)
        consts[:, c] = (gi + 1.0).astype(np.float32)
        consts[:, NCH + c] = (1.0 / (gi + 1.0)).astype(np.float32)
    return cmask, consts


def _build(big_dt=BIG_DT):
    """Build + compile the per-core Bass module (SPMD: same program, 8 cores)."""
    key = big_dt
    if key in _compiled:
        return _compiled[key]

    import concourse.bacc as bacc
    import concourse.bass as bass
    import concourse.mybir as mybir
    import concourse.tile as tile
    from concourse.masks import make_identity

    f32 = mybir.dt.float32
    i32 = mybir.dt.int32
    if big_dt == "f32r":
        mm_dt = mybir.dt.float32r
    elif big_dt == "f32":
        mm_dt = mybir.dt.float32
    elif big_dt == "bf16":
        mm_dt = mybir.dt.bfloat16
    else:
        raise ValueError(big_dt)
    mm_4byte = big_dt in ("f32r", "f32")

    nc = bacc.Bacc(
        "TRN2",
        target_bir_lowering=False,
        debug=False,
        enable_asserts=False,
        num_devices=NCORES,
    )

    bx_d = nc.dram_tensor("bx", [ROWS, H], mm_dt, kind="ExternalInput").ap()
    idx_d = nc.dram_tensor("idx", [C, BPC * NCH], i32, kind="ExternalInput").ap()
    # fused gather table: row v = [ae[v] | (ae @ w)[v]] in bf16 (A = E @ w
    # precomputed on host; one indirect DMA yields both E and A rows per token,
    # and bf16 rows keep the on-device transposes single-pass)
    eaw_d = nc.dram_tensor("eaw", [V, 2 * D], mybir.dt.bfloat16, kind="ExternalInput").ap()
    cm_d = nc.dram_tensor("cmask", [C, NCH * C], f32, kind="ExternalInput").ap()
    ct_d = nc.dram_tensor("consts", [C, 2 * NCH], f32, kind="ExternalInput").ap()
    out_d = nc.dram_tensor("out", [ROWS, H], f32, kind="ExternalOutput").ap()

    mult = mybir.AluOpType.mult
    add = mybir.AluOpType.add

    with tile.TileContext(nc) as tc:
        with (
            tc.tile_pool(name="const", bufs=1) as cpool,
            tc.tile_pool(name="bxp", bufs=6) as bxpool,
            tc.tile_pool(name="outp", bufs=4) as outpool,
            tc.tile_pool(name="sm", bufs=4) as smpool,
            tc.tile_pool(name="eap", bufs=6) as eapool,
            tc.tile_pool(name="mp", bufs=2) as mpool,
            tc.tile_pool(name="ps_et", bufs=1, space="PSUM") as ps_et,
            tc.tile_pool(name="ps_at", bufs=1, space="PSUM") as ps_at,
            tc.tile_pool(name="ps_s", bufs=2, space="PSUM") as ps_s,
            tc.tile_pool(name="ps_out", bufs=1, space="PSUM") as ps_out,
            tc.tile_pool(name="ps_m", bufs=1, space="PSUM") as ps_m,
        ):
            ident16 = cpool.tile([C, C], mybir.dt.bfloat16)
            make_identity(nc, ident16[:])
            # idx + consts first: every gather waits on idx_s, so it must not
            # queue behind the 1MB cmask on the sync DMA FIFO
            idx_s = cpool.tile([C, BPC * NCH], i32)
            nc.sync.dma_start(out=idx_s[:], in_=idx_d[:, :])
            consts_s = cpool.tile([C, 2 * NCH], f32)
            nc.sync.dma_start(out=consts_s[:], in_=ct_d[:, :])
            cmask_s = cpool.tile([C, NCH * C], f32)
            nc.sync.dma_start(out=cmask_s[:, 0:C], in_=cm_d[:, 0:C])
            nc.sync.dma_start(out=cmask_s[:, C:], in_=cm_d[:, C:])

            for b in range(BPC):
                M_p = ps_m.tile([D, H], f32, name=f"M_p_b{b}", tag="M_p")
                for c in range(NCH):
                    g = b * NCH + c
                    rows = slice(g * C, (g + 1) * C)

                    # one DMA loads two chunks' bx (fewer queue-issue slots,
                    # bigger transfers): [256, H] -> [128, 2H] side by side
                    if c % 2 == 0:
                        BX2 = bxpool.tile([C, 2 * H], mm_dt, name="BX2", tag="BX2")
                        nc.sync.dma_start(
                            out=BX2[:].rearrange("p (two h) -> p two h", two=2),
                            in_=bx_d[g * C:(g + 2) * C, :].rearrange(
                                "(two p) h -> p two h", two=2
                            ),
                        )
                    BX = BX2[:, :H] if c % 2 == 0 else BX2[:, H:]

                    if c > 0:
                        M_s = mpool.tile([D, H], mm_dt, name="M_s", tag="M_s")
                        nc.scalar.copy(out=M_s[:], in_=M_p[:])

                    EA = eapool.tile([C, 2 * D], mybir.dt.bfloat16, name="EA", tag="EA")
                    nc.gpsimd.indirect_dma_start(
                        out=EA[:],
                        out_offset=None,
                        in_=eaw_d[:, :],
                        in_offset=bass.IndirectOffsetOnAxis(
                            ap=idx_s[:, g:g + 1], axis=0
                        ),
                    )

                    et_p = ps_et.tile([D, C], mm_dt, name="et_p", tag="et_p")
                    at_p = ps_at.tile([D, C], mm_dt, name="at_p", tag="at_p")
                    et_v = et_p[:]
                    at_v = at_p[:]
                    nc.tensor.transpose(
                        out=et_v, in_=EA[:, 0:D], identity=ident16[:]
                    )
                    nc.tensor.transpose(
                        out=at_v, in_=EA[:, D:2 * D], identity=ident16[:]
                    )
                    Et = smpool.tile([D, C], mm_dt, name="Et", tag="Et")
                    nc.scalar.copy(out=Et[:], in_=et_v)
                    At = smpool.tile([D, C], mm_dt, name="At", tag="At")
                    nc.scalar.copy(out=At[:], in_=at_v)

                    # Ap = A * (i+1)  [C, D]   (row i = (i+1) a_i)
                    Ap = smpool.tile([C, D], mm_dt, name="Ap", tag="Ap")
                    nc.vector.tensor_scalar_mul(
                        out=Ap[:], in0=EA[:, D:2 * D], scalar1=consts_s[:, c:c + 1]
                    )

                    # S = At^T @ Et  [C, C];  St = S * cmask_c
                    s_p = ps_s.tile([C, C], f32, name="s_p", tag="s_p")
                    nc.tensor.matmul(
                        out=s_p[:], lhsT=At[:], rhs=Et[:], start=True, stop=True,
                    )
                    St = smpool.tile([C, C], mm_dt, name="St", tag="St")
                    nc.vector.tensor_tensor(
                        out=St[:],
                        in0=s_p[:],
                        in1=cmask_s[:, c * C:(c + 1) * C],
                        op=mult,
                    )

                    # M += Ap^T @ BX  [D, H]  (skip the never-read last update).
                    # skip_group_check: the sim's group guard can't express this
                    # read-between-accumulations pattern; the pending-zero
                    # accumulate semantics and Tile's HW sync are unaffected.
                    if c < NCH - 1:
                        for lo, hi in ((0, 512), (512, H)):
                            nc.tensor.matmul(
                                out=M_p[:, lo:hi],
                                lhsT=Ap[:],
                                rhs=BX[:, lo:hi],
                                start=(c == 0),
                                stop=True,
                                skip_group_check=True,
                            )

                    # acc = St^T @ BX (+ Et^T @ M)  [C, H]
                    out_p = ps_out.tile([C, H], f32, name="out_p", tag="out_p")
                    for lo, hi in ((0, 512), (512, H)):
                        nc.tensor.matmul(
                            out=out_p[:, lo:hi],
                            lhsT=St[:],
                            rhs=BX[:, lo:hi],
                            start=True,
                            stop=(c == 0),
                        )
                        if c > 0:
                            nc.tensor.matmul(
                                out=out_p[:, lo:hi],
                                lhsT=Et[:],
                                rhs=M_s[:, lo:hi],
                                start=False,
                                stop=True,
                            )


                    # out = acc * (1/(j+1)) + bx
                    if c % 2 == 0:
                        OUT2 = outpool.tile([C, 2 * H], f32, name="OUT2", tag="OUT2")
                    out_s = OUT2[:, :H] if c % 2 == 0 else OUT2[:, H:]
                    nc.vector.scalar_tensor_tensor(
                        out=out_s,
                        in0=out_p[:],
                        scalar=consts_s[:, NCH + c:NCH + c + 1],
                        in1=BX[:, :].bitcast(f32) if mm_4byte else BX[:, :],
                        op0=mult,
                        op1=add,
                    )
                    if c % 2 == 1:
                        nc.sync.dma_start(
                            out=out_d[(g - 1) * C:(g + 1) * C, :].rearrange(
                                "(two p) h -> p two h", two=2
                            ),
                            in_=OUT2[:].rearrange("p (two h) -> p two h", two=2),
                        )

    nc.compile()
    _compiled[key] = nc
    return nc


def _in_maps(bert_x, x, ae, w, big_dt=BIG_DT):
    import ml_dtypes

    host_mm = np.float32 if big_dt in ("f32r", "f32") else ml_dtypes.bfloat16
    bert_x = np.ascontiguousarray(np.asarray(bert_x, dtype=np.float32).astype(host_mm))
    x = np.asarray(x)
    ae = np.asarray(ae, dtype=np.float32)
    w = np.asarray(w, dtype=np.float32)
    eaw = np.ascontiguousarray(
        np.concatenate([ae, ae @ w], axis=1).astype(ml_dtypes.bfloat16)
    )
    cmask, consts = _np_consts()
    # idx layout: [C, BPC*NCH] int32, column b*NCH+c = chunk c of local batch b
    xr = x.reshape(B, NCH, C).transpose(0, 2, 1).astype(np.int32)  # [B, C, NCH]
    maps = []
    for k in range(NCORES):
        maps.append(
            {
                "bx": bert_x[k * BPC:(k + 1) * BPC].reshape(ROWS, H),
                "idx": np.ascontiguousarray(
                    np.concatenate([xr[k * BPC + b] for b in range(BPC)], axis=1)
                ),
                "eaw": eaw,
                "cmask": cmask,
                "consts": consts,
            }
        )
    return maps


def _run(bert_x, x, ae, w, trace=False, big_dt=BIG_DT):
    from concourse import bass_utils

    nc = _build(big_dt)
    maps = _in_maps(bert_x, x, ae, w, big_dt)
    res = bass_utils.run_bass_kernel_spmd(
        nc, maps, core_ids=list(range(NCORES)), trace=trace
    )
    out = np.concatenate(
        [res.results[k]["out"].reshape(BPC, L, H) for k in range(NCORES)], axis=0
    )
    return out, res


def kernel(bert_x, x, ae, w):
    out, _ = _run(bert_x, x, ae, w, trace=False)
    return out


# revision 44
# speedup vs baseline: 1.0731x; 1.0731x over previous
"""Trainium2 Bass kernel for nn_CausalLayer (bilinear causal mixing layer).

Math (per batch b):
    E = ae[x]                                # [L, D] gather
    S[i,j] = E_i @ w @ E_j                   # bilinear pairwise score
    coef[i,j] = (i+1)/(j+1) for i<j else 0
    res[:,j] = bx[:,j] + sum_i coef[i,j]*S[i,j]*bx[:,i]

Rather than materializing the [L, L] score matrix (O(L^2 H) flops), we use the
chunked linear-attention identity. With a_i = w^T E_i and y_i = (i+1)*bx_i:

    res_j = bx_j + (1/(j+1)) * [ M_cj @ E_j + sum_{i<j, same chunk} (a_i.E_j) y_i ]
    M_c   = sum_{i in chunks < c} y_i a_i^T      (rank-D running state, [D, H])

Per 128-token chunk that is: a few tiny [*,64/128] matmuls, one masked [128,128]
score block, and three [*,768] matmuls -- O(L*C*(D+H) + L*D*H) total, 16x fewer
flops than the reference einsum, which puts the kernel at the HBM roofline
(bf16 bx in + f32 res out + gathers ~= 21 MB/core).

Sharding: batch-parallel, 2 of 16 batches per NeuronCore across 8 cores; ae/w
and the small constant tables are replicated. No cross-core communication.
"""

import os
import sys

for _p in ("/opt/trn_rl_repo", "/root/.axon_site/_ro/trn_rl_repo"):
    if os.path.isdir(_p) and _p not in sys.path:
        sys.path.insert(0, _p)

import numpy as np

B, L, H = 16, 2048, 768
V, D = 30000, 64
NCORES = 8
BPC = B // NCORES          # batches per core
C = 128                    # chunk (tile) size along sequence
NCH = L // C               # chunks per batch
ROWS = BPC * L             # bx rows per core

# dtype for the matmul path. This build is tuned for "bf16" (the fused gather
# table and transposes are bf16); measured scale-relative absmax error vs the
# fp32 reference is ~3.3e-3 with fp32 PSUM accumulation throughout.
BIG_DT = "bf16"

_compiled = {}


def _np_consts():
    i = np.arange(C, dtype=np.float64)
    cmask = np.zeros((C, NCH * C), np.float32)
    consts = np.zeros((C, 2 * NCH), np.float32)
    for c in range(NCH):
        gi = c * C + i
        cmask[:, c * C:(c + 1) * C] = np.where(
            i[:, None] < i[None, :], (gi + 1.0)[:, None], 0.0
        ).astype(np.float32)
        consts[:, c] = (gi + 1.0).astype(np.float32)
        consts[:, NCH + c] = (1.0 / (gi + 1.0)).astype(np.float32)
    return cmask, consts


def _build(big_dt=BIG_DT):
    """Build + compile the per-core Bass module (SPMD: same program, 8 cores)."""
    key = big_dt
    if key in _compiled:
        return _compiled[key]

    import concourse.bacc as bacc
    import concourse.bass as bass
    import concourse.mybir as mybir
    import concourse.tile as tile
    from concourse.masks import make_identity

    f32 = mybir.dt.float32
    i32 = mybir.dt.int32
    if big_dt == "f32r":
        mm_dt = mybir.dt.float32r
    elif big_dt == "f32":
        mm_dt = mybir.dt.float32
    elif big_dt == "bf16":
        mm_dt = mybir.dt.bfloat16
    else:
        raise ValueError(big_dt)
    mm_4byte = big_dt in ("f32r", "f32")

    nc = bacc.Bacc(
        "TRN2",
        target_bir_lowering=False,
        debug=False,
        enable_asserts=False,
        num_devices=NCORES,
    )

    bx_d = nc.dram_tensor("bx", [ROWS, H], mm_dt, kind="ExternalInput").ap()
    idx_d = nc.dram_tensor("idx", [C, BPC * NCH], i32, kind="ExternalInput").ap()
    # fused gather table: row v = [ae[v] | (ae @ w)[v]] in bf16 (A = E @ w
    # precomputed on host; one indirect DMA yields both E and A rows per token,
    # and bf16 rows keep the on-device transposes single-pass)
    eaw_d = nc.dram_tensor("eaw", [V, 2 * D], mybir.dt.bfloat16, kind="ExternalInput").ap()
    cm_d = nc.dram_tensor("cmask", [C, NCH * C], f32, kind="ExternalInput").ap()
    ct_d = nc.dram_tensor("consts", [C, 2 * NCH], f32, kind="ExternalInput").ap()
    out_d = nc.dram_tensor("out", [ROWS, H], f32, kind="ExternalOutput").ap()

    mult = mybir.AluOpType.mult
    add = mybir.AluOpType.add

    with tile.TileContext(nc) as tc:
        with (
            tc.tile_pool(name="const", bufs=1) as cpool,
            tc.tile_pool(name="bxp", bufs=6) as bxpool,
            tc.tile_pool(name="outp", bufs=4) as outpool,
            tc.tile_pool(name="sm", bufs=4) as smpool,
            tc.tile_pool(name="eap", bufs=6) as eapool,
            tc.tile_pool(name="mp", bufs=2) as mpool,
            tc.tile_pool(name="ps_et", bufs=1, space="PSUM") as ps_et,
            tc.tile_pool(name="ps_at", bufs=1, space="PSUM") as ps_at,
            tc.tile_pool(name="ps_s", bufs=2, space="PSUM") as ps_s,
            tc.tile_pool(name="ps_out", bufs=1, space="PSUM") as ps_out,
            tc.tile_pool(name="ps_m", bufs=1, space="PSUM") as ps_m,
        ):
            ident16 = cpool.tile([C, C], mybir.dt.bfloat16)
            make_identity(nc, ident16[:])
            # idx + consts first: every gather waits on idx_s, so it must not
            # queue behind the 1MB cmask on the sync DMA FIFO
            idx_s = cpool.tile([C, BPC * NCH], i32)
            nc.sync.dma_start(out=idx_s[:], in_=idx_d[:, :])
            consts_s = cpool.tile([C, 2 * NCH], f32)
            nc.sync.dma_start(out=consts_s[:], in_=ct_d[:, :])
            cmask_s = cpool.tile([C, NCH * C], f32)
            nc.sync.dma_start(out=cmask_s[:, 0:C], in_=cm_d[:, 0:C])
            nc.sync.dma_start(out=cmask_s[:, C:], in_=cm_d[:, C:])

            for b in range(BPC):
                M_p = ps_m.tile([D, H], f32, name=f"M_p_b{b}", tag="M_p")
                for c in range(NCH):
                    g = b * NCH + c
                    rows = slice(g * C, (g + 1) * C)

                    # one DMA loads two chunks' bx (fewer queue-issue slots,
                    # bigger transfers): [256, H] -> [128, 2H] side by side
                    if c % 2 == 0:
                        BX2 = bxpool.tile([C, 2 * H], mm_dt, name="BX2", tag="BX2")
                        nc.sync.dma_start(
                            out=BX2[:].rearrange("p (two h) -> p two h", two=2),
                            in_=bx_d[g * C:(g + 2) * C, :].rearrange(
                                "(two p) h -> p two h", two=2
                            ),
                        )
                    BX = BX2[:, :H] if c % 2 == 0 else BX2[:, H:]

                    if c > 0:
                        M_s = mpool.tile([D, H], mm_dt, name="M_s", tag="M_s")
                        nc.scalar.copy(out=M_s[:], in_=M_p[:])

                    EA = eapool.tile([C, 2 * D], mybir.dt.bfloat16, name="EA", tag="EA")
                    nc.gpsimd.indirect_dma_start(
                        out=EA[:],
                        out_offset=None,
                        in_=eaw_d[:, :],
                        in_offset=bass.IndirectOffsetOnAxis(
                            ap=idx_s[:, g:g + 1], axis=0
                        ),
                    )

                    et_p = ps_et.tile([D, C], mm_dt, name="et_p", tag="et_p")
                    at_p = ps_at.tile([D, C], mm_dt, name="at_p", tag="at_p")
                    et_v = et_p[:]
                    at_v = at_p[:]
                    nc.tensor.transpose(
                        out=et_v, in_=EA[:, 0:D], identity=ident16[:]
                    )
                    nc.tensor.transpose(
                        out=at_v, in_=EA[:, D:2 * D], identity=ident16[:]
                    )
                    Et = smpool.tile([D, C], mm_dt, name="Et", tag="Et")
                    nc.scalar.copy(out=Et[:], in_=et_v)
                    At = smpool.tile([D, C], mm_dt, name="At", tag="At")
                    nc.scalar.copy(out=At[:], in_=at_v)

                    # Ap = A * (i+1)  [C, D]   (row i = (i+1) a_i)
                    Ap = smpool.tile([C, D], mm_dt, name="Ap", tag="Ap")
                    nc.vector.tensor_scalar_mul(
                        out=Ap[:], in0=EA[:, D:2 * D], scalar1=consts_s[:, c:c + 1]
                    )

                    # S = At^T @ Et  [C, C];  St = S * cmask_c
                    s_p = ps_s.tile([C, C], f32, name="s_p", tag="s_p")
                    nc.tensor.matmul(
                        out=s_p[:], lhsT=At[:], rhs=Et[:], start=True, stop=True,
                    )
                    St = smpool.tile([C, C], mm_dt, name="St", tag="St")
                    nc.vector.tensor_tensor(
                        out=St[:],
                        in0=s_p[:],
                        in1=cmask_s[:, c * C:(c + 1) * C],
                        op=mult,
                    )

                    # M += Ap^T @ BX  [D, H]  (skip the never-read last update).
                    # skip_group_check: the sim's group guard can't express this
                    # read-between-accumulations pattern; the pending-zero
                    # accumulate semantics and Tile's HW sync are unaffected.
                    if c < NCH - 1:
                        for lo, hi in ((0, 512), (512, H)):
                            nc.tensor.matmul(
                                out=M_p[:, lo:hi],
                                lhsT=Ap[:],
                                rhs=BX[:, lo:hi],
                                start=(c == 0),
                                stop=True,
                                skip_group_check=True,
                            )

                    # acc = St^T @ BX (+ Et^T @ M)  [C, H]
                    out_p = ps_out.tile([C, H], f32, name="out_p", tag="out_p")
                    for lo, hi in ((0, 512), (512, H)):
                        nc.tensor.matmul(
                            out=out_p[:, lo:hi],
                            lhsT=St[:],
                            rhs=BX[:, lo:hi],
                            start=True,
                            stop=(c == 0),
                        )
                    if c > 0:
                        for lo, hi in ((0, 512), (512, H)):
                            nc.tensor.matmul(
                                out=out_p[:, lo:hi],
                                lhsT=Et[:],
                                rhs=M_s[:, lo:hi],
                                start=False,
                                stop=True,
                            )


                    # out = acc * (1/(j+1)) + bx
                    if c % 2 == 0:
                        OUT2 = outpool.tile([C, 2 * H], f32, name="OUT2", tag="OUT2")
                    out_s = OUT2[:, :H] if c % 2 == 0 else OUT2[:, H:]
                    nc.vector.scalar_tensor_tensor(
                        out=out_s,
                        in0=out_p[:],
                        scalar=consts_s[:, NCH + c:NCH + c + 1],
                        in1=BX[:, :].bitcast(f32) if mm_4byte else BX[:, :],
                        op0=mult,
                        op1=add,
                    )
                    if c % 2 == 1:
                        nc.sync.dma_start(
                            out=out_d[(g - 1) * C:(g + 1) * C, :].rearrange(
                                "(two p) h -> p two h", two=2
                            ),
                            in_=OUT2[:].rearrange("p (two h) -> p two h", two=2),
                        )

    # Adjacent PE matmuls sharing a stationary operand reload it redundantly;
    # mark the second of each such pair as pre-loaded (ldweights=True).
    for blk in nc.m.functions[0].blocks:
        last = None
        for inst in blk.instructions:
            if getattr(inst, "engine", None) != mybir.EngineType.PE:
                continue
            if not isinstance(inst, mybir.InstMatmult):
                if isinstance(inst, (mybir.InstLdweights,)):
                    last = None
                continue
            if (
                last is not None
                and not inst.is_transpose
                and not last.is_transpose
                and inst.ins[1].memref == last.ins[1].memref
                and inst.ins[1].offset == last.ins[1].offset
                and inst.ins[1].ap == last.ins[1].ap
            ):
                inst.ldweights = True
            last = inst

    nc.compile()
    _compiled[key] = nc
    return nc


def _in_maps(bert_x, x, ae, w, big_dt=BIG_DT):
    import ml_dtypes

    host_mm = np.float32 if big_dt in ("f32r", "f32") else ml_dtypes.bfloat16
    bert_x = np.ascontiguousarray(np.asarray(bert_x, dtype=np.float32).astype(host_mm))
    x = np.asarray(x)
    ae = np.asarray(ae, dtype=np.float32)
    w = np.asarray(w, dtype=np.float32)
    eaw = np.ascontiguousarray(
        np.concatenate([ae, ae @ w], axis=1).astype(ml_dtypes.bfloat16)
    )
    cmask, consts = _np_consts()
    # idx layout: [C, BPC*NCH] int32, column b*NCH+c = chunk c of local batch b
    xr = x.reshape(B, NCH, C).transpose(0, 2, 1).astype(np.int32)  # [B, C, NCH]
    maps = []
    for k in range(NCORES):
        maps.append(
            {
                "bx": bert_x[k * BPC:(k + 1) * BPC].reshape(ROWS, H),
                "idx": np.ascontiguousarray(
                    np.concatenate([xr[k * BPC + b] for b in range(BPC)], axis=1)
                ),
                "eaw": eaw,
                "cmask": cmask,
                "consts": consts,
            }
        )
    return maps


def _run(bert_x, x, ae, w, trace=False, big_dt=BIG_DT):
    from concourse import bass_utils

    nc = _build(big_dt)
    maps = _in_maps(bert_x, x, ae, w, big_dt)
    res = bass_utils.run_bass_kernel_spmd(
        nc, maps, core_ids=list(range(NCORES)), trace=trace
    )
    out = np.concatenate(
        [res.results[k]["out"].reshape(BPC, L, H) for k in range(NCORES)], axis=0
    )
    return out, res


def kernel(bert_x, x, ae, w):
    out, _ = _run(bert_x, x, ae, w, trace=False)
    return out
